# revision 1
# baseline (speedup 1.0000x reference)
"""Trainium2 Bass kernel for a causal MHA block with RoPE (nn_CustomMHA).

Full-input contract: kernel(**inputs) takes the complete x/qkv/wo arrays,
shards them across 8 NeuronCores internally (head-group x batch), runs one
SPMD Bass program, and reassembles the full output on the host.

Sharding: core c => head group g = c // 4 (8 of 16 heads), batch b = c % 4.
Each core computes QKV projection + RoPE + causal attention + the wo
projection restricted to its head group's columns; the host sums the two
head-group partial outputs per batch (the "all-reduce" of the tensor-parallel
split) while unsharding.

Layout notes (everything feature-major / transposed so the PE contracts on
partitions):
  xT   [D, S]   = x[b].T          (fp32r)
  Q^T/K^T [512, S] per group      (bf16, RoPE applied)
  V65  [S/128, 128, 8*65]         (fp32r; per-head 65-wide slot, col 64 = 1.0
                                   so attn@V also accumulates the row sums)
  scores^T psum [128 keys, 512 queries] -> exp on ScalarE -> p (fp32r)
  attnout^T [512, S] fp32r -> woT matmuls -> out^T [D, S] partial
"""

import math

import numpy as np

D_MODEL = 1024
N_HEADS = 16
DH = 64
THETA = 10000.0
B_GLOB = 4
S_GLOB = 2048
N_CORES = 8
HG = 8            # heads per core (group)
FG = HG * DH      # feature columns per group = 512
QB = 512          # query block (max fp32 PSUM bank width)
KT = 128          # key tile (psum partitions)


def build_nc(S=S_GLOB, num_devices=N_CORES, reps=1):
    """Build the per-core SPMD Bass program (same program on every core)."""
    import concourse.bacc as bacc
    import concourse.mybir as mybir
    import concourse.tile as tile

    F32 = mybir.dt.float32
    F32R = mybir.dt.float32r
    BF16 = mybir.dt.bfloat16
    Exp = mybir.ActivationFunctionType.Exp

    n_qb = S // QB          # query blocks
    n_st = S // KT          # seq tiles of 128
    n_j = D_MODEL // 128    # contraction tiles over D

    nc = bacc.Bacc("TRN2", target_bir_lowering=False, debug=False,
                   num_devices=num_devices)

    xT = nc.dram_tensor("xT", [D_MODEL, S], F32R, kind="ExternalInput")
    wqk = nc.dram_tensor("wqk", [D_MODEL, 2 * FG], F32R, kind="ExternalInput")
    wv = nc.dram_tensor("wv", [D_MODEL, FG], F32R, kind="ExternalInput")
    woT = nc.dram_tensor("woT", [FG, D_MODEL], F32R, kind="ExternalInput")
    cosT = nc.dram_tensor("cosT", [128, S], F32, kind="ExternalInput")
    sinTs = nc.dram_tensor("sinTs", [128, S], F32, kind="ExternalInput")
    mwide = nc.dram_tensor("mwide", [128, QB + 384], F32, kind="ExternalInput")
    outT = nc.dram_tensor("outT", [D_MODEL, S], F32, kind="ExternalOutput")

    with tile.TileContext(nc) as tc:
        from contextlib import ExitStack
        with ExitStack() as octx:
            if reps > 1:
                octx.enter_context(tc.For_i(0, reps, 1))
            ctx = octx.enter_context(ExitStack())
            persist = ctx.enter_context(tc.tile_pool(name="persist", bufs=1))

            QT_sb = persist.tile([128, FG // 128, S], BF16, tag="QT")
            KT_sb = persist.tile([128, FG // 128, S], BF16, tag="KT")
            V65_sb = persist.tile([128, n_st, HG * 65], F32R, tag="V65")

            # ------- phase 1: projections, seq-chunk streamed --------------
            ctx1 = ExitStack()
            pxT = ctx1.enter_context(tc.tile_pool(name="pxT", bufs=2))
            pw = ctx1.enter_context(tc.tile_pool(name="pw", bufs=1))
            pcs = ctx1.enter_context(tc.tile_pool(name="pcs", bufs=2))
            rope_p = ctx1.enter_context(tc.tile_pool(name="rope_p", bufs=1))
            psum1 = ctx1.enter_context(tc.tile_pool(name="ps1", bufs=4, space="PSUM"))

            wv_sb = pw.tile([128, n_j, FG], F32R, tag="wv")
            wqk_sb = pw.tile([128, n_j, 2 * FG], F32R, tag="wqk")

            for s in range(n_qb):
                scols = slice(s * QB, (s + 1) * QB)
                xTc = pxT.tile([128, n_j, QB], F32R, tag="xTc")
                for j in range(n_j):
                    nc.sync.dma_start(xTc[:, j, :], xT[j * 128:(j + 1) * 128, scols])
                    if s == 0:
                        nc.sync.dma_start(wv_sb[:, j, :], wv[j * 128:(j + 1) * 128, :])
                cs2 = pcs.tile([128, 2, QB], F32, tag="cs2")
                nc.sync.dma_start(cs2[:, 0, :], cosT[:, scols])
                nc.sync.dma_start(cs2[:, 1, :], sinTs[:, scols])
                if s == 0:
                    for j in range(n_j):
                        nc.sync.dma_start(wqk_sb[:, j, :],
                                          wqk[j * 128:(j + 1) * 128, :])

                # V projection for this chunk's 4 seq tiles
                for st in range(4 * s, 4 * s + 4):
                    nc.vector.memset(V65_sb[:, st, :].bitcast(F32), 1.0)
                    ps = psum1.tile([128, FG], F32, tag="pv")
                    for j in range(n_j):
                        nc.tensor.matmul(
                            ps[:],
                            xTc[:, j, (st - 4 * s) * 128:(st - 4 * s + 1) * 128],
                            wv_sb[:, j, :],
                            start=(j == 0), stop=(j == n_j - 1))
                    nc.vector.tensor_copy(
                        V65_sb[:, st, :].rearrange("p (h x) -> p h x", x=65)[:, :, 0:64],
                        ps[:].rearrange("p (h x) -> p h x", x=64))

                # Q^T / K^T projection + RoPE for this chunk; sin-products
                # are collected for all 8 m-tiles so the partition swap is
                # 4 large DMAs per chunk instead of 32 small ones.
                t2all = rope_p.tile([128, 2 * FG // 128, QB], F32, tag="t2all")
                t2wall = rope_p.tile([128, 2 * FG // 128, QB], F32, tag="t2wall")
                t1all = rope_p.tile([128, 2 * FG // 128, QB], F32, tag="t1all")
                for m in range(2 * FG // 128):
                    ps = psum1.tile([128, QB], F32, tag="pp")
                    for j in range(n_j):
                        nc.tensor.matmul(
                            ps[:],
                            wqk_sb[:, j, m * 128:(m + 1) * 128],
                            xTc[:, j, :],
                            start=(j == 0), stop=(j == n_j - 1))
                    nc.vector.tensor_mul(t1all[:, m, :], ps[:], cs2[:, 0, :])
                    nc.vector.tensor_mul(t2all[:, m, :], ps[:], cs2[:, 1, :])
                    if m == 3 or m == 7:
                        for o in (0, 32, 64, 96):
                            nc.sync.dma_start(
                                t2wall[o:o + 32, m - 3:m + 1, :],
                                t2all[o ^ 32:(o ^ 32) + 32, m - 3:m + 1, :])
                        dst = QT_sb if m < 4 else KT_sb
                        for mm in range(m - 3, m + 1):
                            nc.vector.tensor_add(
                                dst[:, mm % 4, scols],
                                t1all[:, mm, :], t2wall[:, mm, :])
            ctx1.close()   # free xT chunks / weights / psum1

            # ------- phase 2+3: attention (qb outer) + interleaved wo ------
            p23 = ctx.enter_context(tc.tile_pool(name="p23", bufs=1))
            attnT_sb = p23.tile([128, FG // 128, S], F32R, tag="attnT")
            woT_sb = p23.tile([128, FG // 128, D_MODEL], F32R, tag="woT")
            mw_sb = p23.tile([128, QB + 384], F32, tag="mw")
            for j in range(FG // 128):
                nc.sync.dma_start(woT_sb[:, j, :], woT[j * 128:(j + 1) * 128, :])
            nc.sync.dma_start(mw_sb[:], mwide[:])
            with tc.tile_pool(name="p2p", bufs=3) as p2p, \
                 tc.tile_pool(name="ps2", bufs=2, space="PSUM") as psum_s, \
                 tc.tile_pool(name="p2av", bufs=3, space="PSUM") as p2av, \
                 tc.tile_pool(name="ps3", bufs=1, space="PSUM") as psum_o, \
                 tc.tile_pool(name="p3o", bufs=4) as p3o, \
                 tc.tile_pool(name="p2dr", bufs=4, space="DRAM") as p2dr:
                for qb in range(n_qb):
                    n_kt = (qb + 1) * (QB // KT)
                    for h in range(HG):
                        p0 = (h % 2) * 64
                        f = h // 2
                        pav = p2av.tile([65, QB], F32, tag="pav")
                        for kp in range(n_kt // 2):
                            dd = 2 * kp - (qb * (QB // KT))
                            # second diagonal pair: only the upper 256
                            # queries can attend these keys; narrow all work
                            pscr = psum_s.tile([128, 2, QB], F32, tag="ps")
                            for i in range(2):
                                kt = 2 * kp + i
                                qi0 = 128 * (dd + i) if dd >= 0 else 0
                                nc.tensor.matmul(
                                    pscr[:, i, qi0:QB],
                                    KT_sb[p0:p0 + 64, f, kt * KT:(kt + 1) * KT],
                                    QT_sb[p0:p0 + 64, f,
                                          qb * QB + qi0:(qb + 1) * QB],
                                    start=True, stop=True)
                            pt = p2p.tile([128, 2, QB], F32R, tag="pt")
                            if dd >= 0:  # diagonal pair: mask after exp.
                                # Each tile i keeps only its valid query
                                # range [qi0, QB); exp is split per tile so
                                # no stale PSUM is ever read.
                                pe = p2p.tile([128, 2, QB], F32, tag="pe")
                                for i in range(2):
                                    qi0 = 128 * (dd + i)
                                    nc.scalar.activation(
                                        pe[:, i, qi0:QB], pscr[:, i, qi0:QB],
                                        Exp)
                                    nc.vector.tensor_mul(
                                        pt[:, i, qi0:QB], pe[:, i, qi0:QB],
                                        mw_sb[:, 384 - 128 * (dd + i) + qi0:
                                              384 - 128 * (dd + i) + QB])
                            else:
                                nc.scalar.activation(pt[:], pscr[:], Exp)
                            for i in range(2):
                                kt = 2 * kp + i
                                qi0 = 128 * (dd + i) if dd >= 0 else 0
                                nc.tensor.matmul(
                                    pav[:, qi0:QB],
                                    V65_sb[:, kt, h * 65:(h + 1) * 65],
                                    pt[:, i, qi0:QB],
                                    start=(kt == 0), stop=(kt == n_kt - 1))
                        # normalize: out[d, q] = pav[d, q] / pav[64, q]
                        srow = p2p.tile([128, QB], F32, tag="srow")
                        nc.vector.tensor_copy(srow[64:65, :], pav[64:65, :])
                        drow = p2dr.tile([1, QB], F32, tag="drow")
                        nc.sync.dma_start(drow[:], srow[64:65, :])
                        rb = p2p.tile([64, QB], F32, tag="rb")
                        nc.sync.dma_start(rb[:], drow[0:1, :].to_broadcast((64, QB)))
                        rrb = p2p.tile([64, QB], F32, tag="rrb")
                        nc.vector.reciprocal(rrb[:], rb[:])
                        nc.vector.tensor_mul(
                            attnT_sb[p0:p0 + 64, f, qb * QB:(qb + 1) * QB],
                            pav[0:64, :], rrb[:])
                    # wo projection for this seq chunk
                    for m in range(n_j):
                        po = psum_o.tile([128, QB], F32, tag="po")
                        for j in range(FG // 128):
                            nc.tensor.matmul(
                                po[:],
                                woT_sb[:, j, m * 128:(m + 1) * 128],
                                attnT_sb[:, j, qb * QB:(qb + 1) * QB],
                                start=(j == 0), stop=(j == FG // 128 - 1))
                        so = p3o.tile([128, QB], F32, tag="so")
                        nc.vector.tensor_copy(so[:], po[:])
                        nc.sync.dma_start(
                            outT[m * 128:(m + 1) * 128, qb * QB:(qb + 1) * QB], so[:])

    nc.compile()
    return nc


def make_tables(S=S_GLOB):
    """Host-side RoPE tables + diagonal causal mask, in kernel layout."""
    inv_freq = 1.0 / (THETA ** (np.arange(0, DH, 2, dtype=np.float64) / DH))
    ang = np.arange(S, dtype=np.float64)[:, None] * inv_freq[None, :]  # [S, 32]
    cos64 = np.concatenate([np.cos(ang), np.cos(ang)], axis=1)  # [S, 64]
    sin32 = np.sin(ang)                                         # [S, 32]
    p = np.arange(128)
    d = p % 64
    cosT = cos64[:, d].T.astype(np.float32)                     # [128, S]
    sign = np.where(d < 32, -1.0, 1.0)
    sinT_signed = (sign[:, None] * sin32[:, p % 32].T).astype(np.float32)
    sinTs = sinT_signed[p ^ 32, :]                              # swap-folded
    # mwide[r, c] = 1 iff c >= r + 384   (diagonal-tile causal masks)
    r = np.arange(128)[:, None]
    c = np.arange(QB + 384)[None, :]
    mwide = (c >= r + 384).astype(np.float32)
    return np.ascontiguousarray(cosT), np.ascontiguousarray(sinTs), mwide


def make_in_maps(x, qkv, wo, S=S_GLOB):
    """Shard full inputs into one input map per core."""
    x = np.asarray(x, dtype=np.float32)
    qkv = np.asarray(qkv, dtype=np.float32)
    wo = np.asarray(wo, dtype=np.float32)
    cosT, sinTs, mwide = make_tables(S)
    scale = 1.0 / math.sqrt(float(DH))
    in_maps = []
    for c in range(N_CORES):
        g, b = c // 4, c % 4
        rows = slice(g * FG, (g + 1) * FG)
        wq = qkv[0 * D_MODEL:1 * D_MODEL][rows] * scale   # [512, 1024]
        wk = qkv[1 * D_MODEL:2 * D_MODEL][rows]
        wv_ = qkv[2 * D_MODEL:3 * D_MODEL][rows]
        in_maps.append({
            "xT": np.ascontiguousarray(x[b].T),
            "wqk": np.ascontiguousarray(np.concatenate([wq, wk], axis=0).T),
            "wv": np.ascontiguousarray(wv_.T),
            "woT": np.ascontiguousarray(wo[:, rows].T),
            "cosT": cosT,
            "sinTs": sinTs,
            "mwide": mwide,
        })
    return in_maps


def assemble_output(results, S=S_GLOB):
    """Sum head-group partials per batch and transpose back to [B, S, D]."""
    out = np.empty((B_GLOB, S, D_MODEL), dtype=np.float32)
    for b in range(B_GLOB):
        acc = results[b]["outT"] + results[4 + b]["outT"]
        out[b] = acc.T
    return out


_NC_CACHE = {}


def kernel(x, qkv, wo):
    from concourse.bass_utils import run_bass_kernel_spmd
    if "nc" not in _NC_CACHE:
        _NC_CACHE["nc"] = build_nc()
    nc = _NC_CACHE["nc"]
    in_maps = make_in_maps(x, qkv, wo)
    res = run_bass_kernel_spmd(nc, in_maps, list(range(N_CORES)))
    return assemble_output(res.results)



# revision 2
# speedup vs baseline: 1.0181x; 1.0181x over previous
"""Trainium2 Bass kernel v2 for causal MHA with RoPE (nn_CustomMHA).

Same external contract as the baseline kernel.py.  Key differences:

  - QK projection in fp8e4 DoubleRow (contraction 256/pass -> 0.5 cyc/row).
  - Scores matmul in fp8e4 DoubleRow: per-head Q/K tiles padded to 128
    partitions (complementary head half zeroed).  The lhsT second k-tile
    aliases the *adjacent* K key-tile (its products vanish against the
    zeroed second k-tile of the moving Q operand), so only Q stores
    physical zeros.
  - V projection / attn@V / wo in bf16 (fp8 there busts the 2e-2 gate).
  - exp on ScalarE writes bf16 probabilities directly (1/sqrt(dh) folded
    into the activation scale).
  - Softmax denominators broadcast via two tiny PE matmuls instead of a
    DRAM round-trip; elementwise work split across DVE / Pool / Act.

Scales: x8 = fp8(x); wqk8 = fp8(64*wqk); rope tables pre-scaled by 2^-6;
exp(scale = 1/8 * scores).  All descales fold into constants.
"""

import math

import numpy as np

D_MODEL = 1024
N_HEADS = 16
DH = 64
THETA = 10000.0
B_GLOB = 4
S_GLOB = 2048
N_CORES = 8
HG = 8            # heads per core
FG = HG * DH      # 512
QB = 512
KT = 128


def build_nc(S=S_GLOB, num_devices=N_CORES, reps=1):
    import concourse.bacc as bacc
    import concourse.mybir as mybir
    import concourse.tile as tile

    F32 = mybir.dt.float32
    U32 = mybir.dt.uint32
    F32R = mybir.dt.float32r
    BF16 = mybir.dt.bfloat16
    F8 = mybir.dt.float8e4
    Exp = mybir.ActivationFunctionType.Exp
    DR = mybir.MatmulPerfMode.DoubleRow

    n_qb = S // QB
    n_st = S // KT
    n_j = D_MODEL // 128    # 8

    nc = bacc.Bacc("TRN2", target_bir_lowering=False, debug=False,
                   num_devices=num_devices)

    xT8 = nc.dram_tensor("xT8", [D_MODEL, S], F8, kind="ExternalInput")
    xT16 = nc.dram_tensor("xT16", [D_MODEL, S], BF16, kind="ExternalInput")
    wqk8 = nc.dram_tensor("wqk8", [D_MODEL, 2 * FG], F8, kind="ExternalInput")
    wv16 = nc.dram_tensor("wv16", [D_MODEL, FG], BF16, kind="ExternalInput")
    woT16 = nc.dram_tensor("woT16", [FG, D_MODEL], BF16, kind="ExternalInput")
    cosT = nc.dram_tensor("cosT", [128, S], F32, kind="ExternalInput")
    sinTs = nc.dram_tensor("sinTs", [128, S], F32, kind="ExternalInput")
    mwide16 = nc.dram_tensor("mwide16", [128, QB + 384], BF16, kind="ExternalInput")
    outT = nc.dram_tensor("outT", [D_MODEL, S], F32, kind="ExternalOutput")

    with tile.TileContext(nc) as tc:
        from contextlib import ExitStack
        with ExitStack() as octx:
            if reps > 1:
                octx.enter_context(tc.For_i(0, reps, 1))
            ctx = octx.enter_context(ExitStack())
            persist = ctx.enter_context(tc.tile_pool(name="persist", bufs=1))

            # per-head score operands, zero-padded to 128 contraction rows
            Q8_sb = persist.tile([128, HG, n_qb, 2, QB], F8, tag="Q8")
            K8_sb = persist.tile([128, HG, n_st + 1, KT], F8, tag="K8")
            V65_sb = persist.tile([128, n_st, HG, 65], BF16, tag="V65")
            attnT_sb = persist.tile([128, FG // 128, S], BF16, tag="attnT")
            woT_sb = persist.tile([128, FG // 128, D_MODEL], BF16, tag="woT")
            mw_sb = persist.tile([128, QB + 384], BF16, tag="mw")
            wv_sb = persist.tile([128, n_j, FG], BF16, tag="wv")
            wqk_sb = persist.tile([128, n_j, 2 * FG], F8, tag="wqk")
            # zero pads: Q second k-tile, Q complementary head halves,
            # K complementary halves + trailing dummy key tile.
            nc.vector.memset(Q8_sb[:, :, :, 1, :], 0.0)
            nc.gpsimd.memset(Q8_sb[64:128, 0:HG:2, :, 0, :], 0.0)
            nc.gpsimd.memset(Q8_sb[0:64, 1:HG:2, :, 0, :], 0.0)
            nc.vector.memset(K8_sb[64:128, 0:HG:2, :, :], 0.0)
            nc.gpsimd.memset(K8_sb[0:64, 1:HG:2, :, :], 0.0)
            nc.vector.memset(K8_sb[:, :, n_st, :], 0.0)
            nc.vector.memset(V65_sb[:, :, :, 64:65], 1.0)

            pxT = ctx.enter_context(tc.tile_pool(name="pxT", bufs=2))
            pcs = ctx.enter_context(tc.tile_pool(name="pcs", bufs=2))
            rope_p = ctx.enter_context(tc.tile_pool(name="rope_p", bufs=1))
            p2p = ctx.enter_context(tc.tile_pool(name="p2p", bufs=3))
            paq = ctx.enter_context(tc.tile_pool(name="paq", bufs=2))
            psum_s = ctx.enter_context(
                tc.tile_pool(name="ps2", bufs=2, space="PSUM"))
            p2av = ctx.enter_context(
                tc.tile_pool(name="p2av", bufs=2, space="PSUM"))
            pshared = ctx.enter_context(
                tc.tile_pool(name="pshared", bufs=2, space="PSUM"))

            for j in range(FG // 128):
                nc.sync.dma_start(woT_sb[:, j, :], woT16[j * 128:(j + 1) * 128, :])
            nc.sync.dma_start(mw_sb[:], mwide16[:])

            def load_chunk(s):
                scols = slice(s * QB, (s + 1) * QB)
                x8c = pxT.tile([128, n_j, QB], F8, tag="x8c")
                x16c = pxT.tile([128, n_j, QB], BF16, tag="x16c")
                for j in range(n_j):
                    nc.sync.dma_start(x8c[:, j, :], xT8[j * 128:(j + 1) * 128, scols])
                    nc.sync.dma_start(x16c[:, j, :], xT16[j * 128:(j + 1) * 128, scols])
                    if s == 0:
                        nc.sync.dma_start(wv_sb[:, j, :], wv16[j * 128:(j + 1) * 128, :])
                        nc.sync.dma_start(wqk_sb[:, j, :], wqk8[j * 128:(j + 1) * 128, :])
                cs2 = pcs.tile([128, 2, QB], F32, tag="cs2")
                nc.sync.dma_start(cs2[:, 0, :], cosT[:, scols])
                nc.sync.dma_start(cs2[:, 1, :], sinTs[:, scols])
                return x8c, x16c, cs2

            def proj_chunk(s, x8c, x16c, cs2):
                # V projection for this chunk's 4 seq tiles (bf16)
                for st4 in range(4):
                    st = 4 * s + st4
                    ps = pshared.tile([128, FG], F32, tag="po")
                    for j in range(n_j):
                        nc.tensor.matmul(
                            ps[:],
                            x16c[:, j, st4 * 128:(st4 + 1) * 128],
                            wv_sb[:, j, :],
                            start=(j == 0), stop=(j == n_j - 1))
                    nc.scalar.copy(
                        V65_sb[:, st, :, 0:64],
                        ps[:].rearrange("p (h x) -> p h x", x=64))

                # QK projection (fp8 DoubleRow) + RoPE
                t1all = rope_p.tile([128, 8, QB], BF16, tag="t1all")
                t2all = rope_p.tile([128, 8, QB], BF16, tag="t2all")
                t2wall = rope_p.tile([128, 8, QB], BF16, tag="t2wall")
                for m in range(8):
                    pp = pshared.tile([128, QB], F32, tag="po")
                    for j in range(4):
                        nc.tensor.matmul(
                            pp[:],
                            wqk_sb[:, 2 * j:2 * j + 2, m * 128:(m + 1) * 128],
                            x8c[:, 2 * j:2 * j + 2, :],
                            start=(j == 0), stop=(j == 3), perf_mode=DR)
                    nc.vector.tensor_mul(t1all[:, m, :], pp[:], cs2[:, 0, :])
                    nc.vector.tensor_mul(t2all[:, m, :], pp[:], cs2[:, 1, :])
                    if m == 3 or m == 7:
                        for o in (0, 32, 64, 96):
                            nc.sync.dma_start(
                                t2wall[o:o + 32, m - 3:m + 1, :],
                                t2all[o ^ 32:(o ^ 32) + 32, m - 3:m + 1, :])
                        for mm in range(m - 3, m + 1):
                            if m == 3:   # Q features, heads (2mm, 2mm+1)
                                d_ev = Q8_sb[0:64, 2 * mm, s, 0, :]
                                d_od = Q8_sb[64:128, 2 * mm + 1, s, 0, :]
                            else:        # K features
                                mk = mm - 4
                                d_ev = K8_sb[0:64, 2 * mk, 4 * s:4 * s + 4, :]
                                d_od = K8_sb[64:128, 2 * mk + 1, 4 * s:4 * s + 4, :]
                            nc.gpsimd.tensor_add(d_ev, t1all[0:64, mm, :],
                                                 t2wall[0:64, mm, :])
                            nc.gpsimd.tensor_add(d_od, t1all[64:128, mm, :],
                                                 t2wall[64:128, mm, :])

            def attn_block(qb):
                n_kt = (qb + 1) * (QB // KT)
                attnQ = paq.tile([128, 4, FG], BF16, tag="attnQ")
                for h in range(HG):
                    pq = p2av.tile([128, 4, 65], F32, tag="pav")
                    for kp in range(n_kt // 2):
                        dd = 2 * kp - (qb * (QB // KT))
                        pscr = psum_s.tile([128, 2, QB], F32, tag="ps")
                        for i in range(2):
                            kt = 2 * kp + i
                            qi0 = 128 * (dd + i) if dd >= 0 else 0
                            nc.tensor.matmul(
                                pscr[:, i, qi0:QB],
                                K8_sb[:, h, kt:kt + 2, :],
                                Q8_sb[:, h, qb, :, qi0:QB],
                                start=True, stop=True, perf_mode=DR)
                        pt = p2p.tile([128, 2, QB], BF16, tag="pt")
                        if dd >= 0:
                            pe_t = p2p.tile([128, 2, QB], BF16, tag="pe")
                            for i in range(2):
                                qi0 = 128 * (dd + i)
                                nc.scalar.activation(
                                    pe_t[:, i, qi0:QB], pscr[:, i, qi0:QB],
                                    Exp, scale=0.125)
                                nc.gpsimd.tensor_mul(
                                    pt[:, i, qi0:QB], pe_t[:, i, qi0:QB],
                                    mw_sb[:, 384 - 128 * (dd + i) + qi0:
                                          384 - 128 * (dd + i) + QB])
                        else:
                            nc.scalar.activation(pt[:], pscr[:], Exp,
                                                 scale=0.125)
                        # attn@V transposed: out[q, d] per 128-query subtile
                        for i in range(2):
                            kt = 2 * kp + i
                            r = kt - 4 * qb
                            for qt in range(max(0, r), 4):
                                nc.tensor.matmul(
                                    pq[:, qt, :],
                                    pt[:, i, qt * 128:(qt + 1) * 128],
                                    V65_sb[:, kt, h, :],
                                    start=(kt == 0 and qt == 0),
                                    stop=(kt == n_kt - 1 and qt == 3))
                    # normalize: denominators live per-partition now
                    rr4 = p2p.tile([128, 4, 1], F32, tag="rr4")
                    nc.vector.reciprocal(rr4[:], pq[:, :, 64:65])
                    for qt in range(4):
                        nc.vector.tensor_scalar_mul(
                            attnQ[:, qt, h * 64:(h + 1) * 64],
                            pq[:, qt, 0:64], rr4[:, qt, :])
                # transpose [q, f] -> [f, q] via DMA xbar
                for qt in range(4):
                    nc.sync.dma_start_transpose(
                        attnT_sb[:, :, qb * QB + qt * 128:
                                 qb * QB + (qt + 1) * 128],
                        attnQ[:, qt, :])

            def wo_block(qb):
                qcols = slice(qb * QB, (qb + 1) * QB)
                for m in range(n_j):
                    po = pshared.tile([128, QB], F32, tag="po")
                    for j in range(FG // 128):
                        nc.tensor.matmul(
                            po[:],
                            woT_sb[:, j, m * 128:(m + 1) * 128],
                            attnT_sb[:, j, qcols],
                            start=(j == 0), stop=(j == FG // 128 - 1))
                    so = p2p.tile([128, QB], F32, tag="so")
                    if m % 2 == 0:
                        nc.scalar.copy(so[:], po[:])
                    else:
                        nc.vector.tensor_copy(so[:], po[:])
                    nc.sync.dma_start(
                        outT[m * 128:(m + 1) * 128, qcols], so[:])

            chunk = load_chunk(0)
            for s in range(n_qb):
                nxt = load_chunk(s + 1) if s + 1 < n_qb else None
                proj_chunk(s, *chunk)
                if s > 0:
                    wo_block(s - 1)
                attn_block(s)
                chunk = nxt
            wo_block(n_qb - 1)

    nc.compile()
    return nc


def make_tables(S=S_GLOB):
    """RoPE tables (pre-scaled by 2^-6 for the fp8 weight scale) and the
    diagonal causal mask."""
    inv_freq = 1.0 / (THETA ** (np.arange(0, DH, 2, dtype=np.float64) / DH))
    ang = np.arange(S, dtype=np.float64)[:, None] * inv_freq[None, :]  # [S, 32]
    cos64 = np.concatenate([np.cos(ang), np.cos(ang)], axis=1)  # [S, 64]
    sin32 = np.sin(ang)                                         # [S, 32]
    p = np.arange(128)
    d = p % 64
    cosT = (cos64[:, d].T / 64.0).astype(np.float32)            # [128, S]
    sign = np.where(d < 32, -1.0, 1.0)
    sinT_signed = (sign[:, None] * sin32[:, p % 32].T / 64.0).astype(np.float32)
    sinTs = sinT_signed[p ^ 32, :]                              # swap-folded
    r = np.arange(128)[:, None]
    c = np.arange(QB + 384)[None, :]
    mwide = (c >= r + 384).astype(np.float32)
    return np.ascontiguousarray(cosT), np.ascontiguousarray(sinTs), mwide


def make_in_maps(x, qkv, wo, S=S_GLOB):
    import ml_dtypes
    F8 = ml_dtypes.float8_e4m3
    BF = ml_dtypes.bfloat16
    x = np.asarray(x, dtype=np.float32)
    qkv = np.asarray(qkv, dtype=np.float32)
    wo = np.asarray(wo, dtype=np.float32)
    cosT, sinTs, mwide = make_tables(S)
    in_maps = []
    for c in range(N_CORES):
        g, b = c // 4, c % 4
        rows = slice(g * FG, (g + 1) * FG)
        wq = qkv[0 * D_MODEL:1 * D_MODEL][rows]           # [512, 1024]
        wk = qkv[1 * D_MODEL:2 * D_MODEL][rows]
        wv_ = qkv[2 * D_MODEL:3 * D_MODEL][rows]
        xT = np.ascontiguousarray(x[b].T)
        wqk = np.concatenate([wq, wk], axis=0).T          # [1024, 1024]
        in_maps.append({
            "xT8": xT.astype(F8),
            "xT16": xT.astype(BF),
            "wqk8": np.ascontiguousarray(wqk * 64.0).astype(F8),
            "wv16": np.ascontiguousarray(wv_.T).astype(BF),
            "woT16": np.ascontiguousarray(wo[:, rows].T).astype(BF),
            "cosT": cosT,
            "sinTs": sinTs,
            "mwide16": mwide.astype(BF),
        })
    return in_maps


def assemble_output(results, S=S_GLOB):
    out = np.empty((B_GLOB, S, D_MODEL), dtype=np.float32)
    for b in range(B_GLOB):
        acc = results[b]["outT"] + results[4 + b]["outT"]
        out[b] = acc.T
    return out


_NC_CACHE = {}


def kernel(x, qkv, wo):
    from concourse.bass_utils import run_bass_kernel_spmd
    if "nc" not in _NC_CACHE:
        _NC_CACHE["nc"] = build_nc()
    nc = _NC_CACHE["nc"]
    in_maps = make_in_maps(x, qkv, wo)
    res = run_bass_kernel_spmd(nc, in_maps, list(range(N_CORES)))
    return assemble_output(res.results)
